# revision 2
# baseline (speedup 1.0000x reference)
"""AttentivePool Trainium2 kernel.

Reference computation (per batch sample b):
    m[c, w]   = mean_h x[b, c, h, w]                      # H-mean pool
    s[c', w]  = tanh(sum_c W[c, c'] m[c, w] + bias[c'])   # additive attention
    a[w]      = sum_c' s[c', w] proj[c']
    p[w]      = softmax_w(a)                              # over W
    out[b, c] = sum_w p[w] m[c, w]

Strategy: pure data-parallel over B across 8 cores (2 samples/core).
x is 1 GiB; memory-bound on streaming it.

HW-measured DMA pattern findings (8 cores streaming concurrently):
  [C, H, 256] W-chunks (1 KiB strided runs)  252 GB/s/core
  [C, 2048] half-rows (1 MiB DMAs)           304 GB/s/core
  [C, 4096] whole h-rows (2 MiB DMAs,
            16 KiB contiguous per partition) 338 GB/s/core  <- used here
so x is streamed as whole h-rows x[b, :, h, :]. The H-sum is a chain of
DVE tensor_adds (fp32 tensor_tensor = 1 elem/lane/cycle @ 0.96 GHz ->
4.4 us/row-add, under the ~5.9 us DMA pace). All x DMAs ride the SP
(sync) HWDGE ring only: nc.scalar dma dispatch shares the ACT engine's
in-order queue with the tail's tanh/exp, whose semaphore waits would
head-of-line-block later x-DMA dispatches (measured ~7 us win from
keeping ACT compute-only).

Attention tail runs per 512-wide chunk on the resident m tile, software
pipelined into the next sample's add chain as two stages ("head" =
PE matmul + ACT tanh -> PE matmul -> ACT exp, no DVE; "rest" = one fused
DVE scalar_tensor_tensor mul with accum_out emitting the weighted
partial sum), one add apart, so the DVE never stalls on the cross-engine
chain. The last row's add is chunked 8x512 to unlock tail chunks early.
Softmax skips max-subtraction: |a| <= sum|proj| = 12.8, exp safe in f32.
The exp's accum_out collects per-chunk denominators; the denominator
reciprocal chain is emitted between the final tail's head and rest.

Scaling: adds compute H*sum; the host folds 1/H into weight_W and into
the broadcast ones-row, so downstream values come out exact (1/H is a
power of two).
"""

import contextlib
from collections import deque

import numpy as np

import concourse.bacc as bacc
import concourse.tile as tile
from concourse import mybir
from concourse.bass_utils import run_bass_kernel_spmd

B, C, H, W = 16, 128, 32, 4096
N_CORES = 8
BL = B // N_CORES   # batch samples per core
TC = 512            # tail chunk width
NCH = W // TC
F32 = mybir.dt.float32
HW = H * W


def build_bass(bl=BL, loop_reps=1, xbufs=8):
    # Bacc (not plain Bass): its compile() runs generate_event_semaphores,
    # which spills >1-wait sync conditions into EventSemaphore instructions
    # (the TRN2 ISA allows a single wait slot per instruction).
    nc = bacc.Bacc(trn_type="TRN2", dynamic_dma_scratch_size=16384)

    x = nc.dram_tensor("x", [bl, C, HW], F32, kind="ExternalInput")
    # All small parameters packed into one tensor: a single DMA means every
    # PE matmul depends on a single weight semaphore (the Matmult/LdWeights
    # sync slot only fits ONE wait, so fan-in must stay at 1).
    # cols 0:128 = weight_W/H, 128:256 = eye(C), 256 = proj, 257 = bias,
    # row 0 of cols 258:386 = 1/H (ones row for broadcasts).
    wpack = nc.dram_tensor("wpack", [C, 386], F32, kind="ExternalInput")
    out = nc.dram_tensor("out", [bl, C], F32, kind="ExternalOutput")

    with tile.TileContext(nc) as tc:
        with (
            tc.tile_pool(name="singles", bufs=1) as singles,
            tc.tile_pool(name="xp", bufs=xbufs) as xpool,
            tc.tile_pool(name="mp", bufs=1) as mpool,
            tc.tile_pool(name="sqp", bufs=2) as sqpool,
            tc.tile_pool(name="ep", bufs=2) as epool,
            tc.tile_pool(name="accp", bufs=1) as accp,
            tc.tile_pool(name="psp", bufs=2, space="PSUM") as psp,
            tc.tile_pool(name="pbp", bufs=2, space="PSUM") as pbp,
            tc.tile_pool(name="psmall", bufs=2, space="PSUM") as psmall,
        ):
            # wpack rides the ACT ring; x rows (SP ring) start at t=0.
            sb_w = singles.tile([C, 386], F32, tag="wpack")
            nc.scalar.dma_start(out=sb_w, in_=wpack[:, :])
            sb_ww = sb_w[:, 0:C]
            sb_ident = sb_w[:, C:2 * C]
            sb_proj = sb_w[:, 2 * C:2 * C + 1]
            sb_bias = sb_w[:, 2 * C + 1:2 * C + 2]
            sb_ones = sb_w[0:1, 2 * C + 2:3 * C + 2]

            # Dummy matmul so PE observes the wpack DMA semaphore before the
            # main loop; later matmuls then only wait on their data operand.
            scr = psmall.tile([1, 1], F32, tag="small")
            nc.tensor.matmul(scr, sb_proj, sb_bias, start=True, stop=True)

            # Per-(b, chunk) accumulators, each slot written exactly once.
            partials = accp.tile([C, bl, NCH], F32, tag="partials")
            dparts = accp.tile([1, bl, NCH], F32, tag="dparts")

            def tail_head(b, kc, m):
                cs = slice(kc * TC, (kc + 1) * TC)
                # squish = tanh((W/H)^T (H*sum) + bias)
                pst = psp.tile([C, TC], F32, tag="ps", name="pst")
                nc.tensor.matmul(pst, sb_ww, m[:, cs], start=True, stop=True)
                sqt = sqpool.tile([C, TC], F32, tag="sq", name="sqt")
                nc.scalar.activation(
                    out=sqt, in_=pst,
                    func=mybir.ActivationFunctionType.Tanh,
                    bias=sb_bias, scale=1.0,
                )
                # attn chunk = proj^T squish  -> [1, TC]
                pat = psmall.tile([1, TC], F32, tag="small", name="pat")
                nc.tensor.matmul(pat, sb_proj, sqt, start=True, stop=True)
                # exp (softmax numerator); accum_out = chunk sum for denom
                et = epool.tile([1, TC], F32, tag="et", name="et")
                nc.scalar.activation(
                    out=et, in_=pat,
                    func=mybir.ActivationFunctionType.Exp,
                    accum_out=dparts[0:1, b, kc:kc + 1],
                )
                return et

            def tail_rest(b, kc, m, et):
                cs = slice(kc * TC, (kc + 1) * TC)
                # broadcast exp/H to all partitions via PE, then one fused
                # DVE op: m_chunk *= bcast, accum_out = weighted partial.
                pbt = pbp.tile([C, TC], F32, tag="pb", name="pbt")
                nc.tensor.matmul(pbt, sb_ones, et, start=True, stop=True)
                nc.vector.scalar_tensor_tensor(
                    out=m[:, cs], in0=pbt, scalar=1.0, in1=m[:, cs],
                    op0=mybir.AluOpType.mult, op1=mybir.AluOpType.mult,
                    accum_out=partials[:, b, kc:kc + 1],
                )

            # Two-stage tail pipeline: 'head' (PE/ACT only, no DVE) goes
            # one drain-step before 'rest' (the DVE op), so the DVE op
            # never sits at the queue head waiting on the 5-hop PE/ACT
            # chain while adds (which release x-tile slots for the DMA
            # ring) queue behind it.
            def drain_one(pending):
                if not pending:
                    return
                e = pending.popleft()
                if e[0] == "head":
                    _, b, kc, m = e
                    et = tail_head(b, kc, m)
                    pending.appendleft(("rest", b, kc, m, et))
                else:
                    _, b, kc, m, et = e
                    tail_rest(b, kc, m, et)

            loop_cm = (
                tc.For_i(0, loop_reps, 1) if loop_reps > 1
                else contextlib.nullcontext()
            )
            with loop_cm:
                pending = deque()
                for b in range(bl):
                    m = mpool.tile([C, W], F32, tag=f"m{b}")
                    xt0 = None
                    for h in range(H):
                        xt = xpool.tile([C, W], F32, tag="xt")
                        nc.sync.dma_start(
                            out=xt, in_=x[b, :, h * W:(h + 1) * W])
                        if h == 0:
                            xt0 = xt
                            continue
                        if h == 1:
                            nc.vector.tensor_add(out=m, in0=xt0, in1=xt)
                        elif h < H - 1:
                            nc.vector.tensor_add(out=m, in0=m, in1=xt)
                        else:
                            # last row chunked: unlocks tail chunks early
                            for kc in range(NCH):
                                cs = slice(kc * TC, (kc + 1) * TC)
                                nc.vector.tensor_add(
                                    out=m[:, cs], in0=m[:, cs], in1=xt[:, cs])
                        # pipeline previous sample's tail between adds
                        drain_one(pending)
                    for kc in range(NCH):
                        pending.append(("head", b, kc, m))

                # Drain. After the final exp, dparts is complete, so the
                # denominator chain is emitted between the last tail's head
                # and rest — it runs in parallel with the weighted-sum tail
                # instead of serially after it.
                while len(pending) > 1 or pending[0][0] != "rest":
                    drain_one(pending)
                _, lb, lkc, lm, let = pending.popleft()
                drow = accp.tile([1, bl], F32, tag="drow")
                nc.vector.reduce_sum(
                    out=drow, in_=dparts, axis=mybir.AxisListType.X)
                nc.scalar.mul(out=drow, in_=drow, mul=1.0 / H)
                nc.vector.reciprocal(out=drow, in_=drow)  # H/denom
                pdb = psmall.tile([C, bl], F32, tag="pdb")
                # (1/H) ones^T @ (H/denom) = 1/denom bcast to all partitions
                nc.tensor.matmul(pdb, sb_ones, drow, start=True, stop=True)
                tail_rest(lb, lkc, lm, let)

                rescol = accp.tile([C, bl], F32, tag="rescol")
                nc.vector.reduce_sum(
                    out=rescol, in_=partials, axis=mybir.AxisListType.X)
                resn = accp.tile([C, bl], F32, tag="resn")
                nc.vector.tensor_mul(out=resn, in0=rescol, in1=pdb)

                # out[b, c] = resn[c, b]: transpose via matmul with identity.
                pt = psmall.tile([bl, C], F32, tag="small")
                nc.tensor.matmul(pt, resn, sb_ident, start=True, stop=True)
                out_sb = accp.tile([bl, C], F32, tag="out_sb")
                nc.vector.tensor_copy(out=out_sb, in_=pt)
                nc.sync.dma_start(out=out[:, :], in_=out_sb)

    nc.compile()
    return nc


def make_in_maps(x, weight_W, weight_proj, bias, bl=BL, n_cores=N_CORES):
    x = np.ascontiguousarray(np.asarray(x, dtype=np.float32))
    wpack = np.zeros((C, 386), dtype=np.float32)
    wpack[:, 0:C] = np.asarray(weight_W, dtype=np.float32) / np.float32(H)
    wpack[:, C:2 * C] = np.eye(C, dtype=np.float32)
    wpack[:, 2 * C:2 * C + 1] = np.asarray(weight_proj, dtype=np.float32)
    wpack[:, 2 * C + 1:2 * C + 2] = np.asarray(bias, dtype=np.float32)
    wpack[0, 2 * C + 2:3 * C + 2] = 1.0 / np.float32(H)
    return [
        {
            "x": np.ascontiguousarray(
                x[i * bl:(i + 1) * bl].reshape(bl, C, HW)),
            "wpack": wpack,
        }
        for i in range(n_cores)
    ]


_NC_CACHE = {}


def kernel(x, weight_W, weight_proj, bias, **run_kwargs):
    if "nc" not in _NC_CACHE:
        _NC_CACHE["nc"] = build_bass()
    nc = _NC_CACHE["nc"]
    in_maps = make_in_maps(x, weight_W, weight_proj, bias)
    res = None
    for attempt in range(3):
        try:
            res = run_bass_kernel_spmd(
                nc, in_maps, core_ids=list(range(N_CORES)), **run_kwargs)
            break
        except Exception:
            # Transient NRT/device hiccups recover on retry; re-raise if not.
            if attempt == 2:
                raise
    out = np.concatenate([r["out"] for r in res.results], axis=0)
    if run_kwargs:
        kernel.last_results = res
    return out
